# revision 31
# baseline (speedup 1.0000x reference)
"""CAM (channel self-attention) kernel for Trainium2 — 8 NeuronCores, batch-parallel.

Math per batch element b (A = x[b] reshaped [N=4096, C=512]):
    G = A^T A                  [C, C]   (symmetric!)
    P = softmax_rows(G)        [C, C]
    Y = A P                    [N, C]
    out = gamma * Y + x

Sharding: data-parallel over batch — core i handles batch element i.
No cross-core communication needed.

Per-core schedule:
  - DMA x in 1 MiB groups -> A32 (f32, resident), cast to A16 (bf16).
  - Per 128-row chunk k, interleaved to keep the PE HAM-warm:
    cast -> 4 PE transposes (A^T blocks -> PSUM -> one strided copy to
    AT16) -> upper-triangle Gram matmuls (free dims 512/384/256/128,
    exploiting G's symmetry).
  - Lower triangle of G reconstructed with 6 PE transposes of the upper
    blocks after the Gram accumulation lands in SBUF.
  - softmax: DVE row-max (negated) -> ACT exp with fused row-sum -> DVE
    reciprocal -> DVE per-row scale, output bf16 P16.
  - Y = A P via PE: lhsT = AT16 tile, rhs = P16.
  - epilogue: one DVE scalar_tensor_tensor: out = (Y * gamma) + A32,
    staged in 512 KiB groups, DMA'd out.
"""

import numpy as np

import concourse.tile as tile
from concourse import bacc, mybir
from concourse.bass_utils import run_bass_kernel_spmd
from concourse.masks import make_identity

B = 8
H = 64
W = 64
C = 512
HW = H * W            # 4096 rows per batch element
NT = HW // 128        # 32 row chunks of 128
CT = C // 128         # 4 col chunks of 128
GRP = 4               # row chunks per input DMA group (1 MiB)
OGRP = 2              # row chunks per output DMA group (512 KiB)
ONG = NT // OGRP      # 16 output groups

F32 = mybir.dt.float32
BF16 = mybir.dt.bfloat16

_CACHE = {}


def _emit(nc, tc, out, x, gamma):
    from contextlib import ExitStack

    with ExitStack() as ctx:
        big = ctx.enter_context(tc.tile_pool(name="big", bufs=1))
        small = ctx.enter_context(tc.tile_pool(name="small", bufs=1))
        stat = ctx.enter_context(tc.tile_pool(name="stat", bufs=4))
        ostage = ctx.enter_context(tc.tile_pool(name="ostage", bufs=4))
        gps = ctx.enter_context(tc.tile_pool(name="gps", bufs=1, space="PSUM"))
        wps = ctx.enter_context(tc.tile_pool(name="wps", bufs=5, space="PSUM"))

        A32 = big.tile([128, NT, C], F32)     # x rows, n on partitions
        A16 = big.tile([128, NT, C], BF16)    # bf16 cast of A32
        AT16 = big.tile([128, CT, HW], BF16)  # A^T, c on partitions
        G32 = big.tile([128, CT, C], F32)     # full Gram matrix in SBUF
        E32 = big.tile([128, CT, C], F32)     # exp(G - rowmax)
        P16 = big.tile([128, CT, C], BF16)    # softmax(G) in bf16

        ident = small.tile([128, 128], BF16)
        make_identity(nc, ident[:])
        ident32 = small.tile([128, 128], F32)
        make_identity(nc, ident32[:])

        gB = small.tile([128, 1], F32)        # gamma broadcast to all partitions

        # PE warm-up: the HAM clock gate holds the PE at 1.2 GHz until it has
        # been busy ~3.4us. The PE is otherwise idle until the first input
        # chunk lands (~11us), so burn that window with dummy matmuls on a
        # zeroed scratch tile; real matmuls then start at 2.4 GHz.
        warm_src = small.tile([128, C], BF16)
        nc.gpsimd.memset(warm_src[:], 0.0)
        warm_ps = wps.tile([128, C], F32, name="warm", tag="w")
        for wi in range(30):
            nc.tensor.matmul(
                warm_ps[:], warm_src[:, 0:128], warm_src[:],
                start=(wi == 0), stop=(wi == 29),
            )

        # Upper-triangle Gram accumulators: G[mi-chunk, mi*128:].
        # g1 (384 cols) and g3 (128 cols) share one PSUM bank.
        g0 = gps.tile([128, C], F32, name="g0", tag="g0")
        g13 = gps.tile([128, C], F32, name="g13", tag="g13")
        g2 = gps.tile([128, C - 256], F32, name="g2", tag="g2")
        g_ps = [g0[:], g13[:, 0:384], g2[:], g13[:, 384:512]]

        # First loads chunk-granular so the PE can start early, then 1 MiB.
        load_groups = [1, 1, 2] + [GRP] * ((NT - 4) // GRP)
        assert sum(load_groups) == NT
        k0 = 0
        for gi, gsz in enumerate(load_groups):
            r0 = k0 * 128
            r1 = (k0 + gsz) * 128
            nc.sync.dma_start(
                A32[:, k0:k0 + gsz, :],
                x[r0:r1, :].rearrange("(t p) c -> p t c", p=128),
            )
            if gi == 0:
                # gamma: tiny load on the ACT HWDGE ring, off the input path
                nc.scalar.dma_start(gB[:], gamma[:])
            for j in range(gsz):
                k = k0 + j
                # cast f32 -> bf16 (DVE; keeps ACT free for A^T copies)
                nc.vector.tensor_copy(A16[:, k, :], A32[:, k, :])
                # A^T blocks of this chunk -> one PSUM bank, one strided copy
                tp = wps.tile([128, CT * 128], BF16, name="tp", tag="w")
                for ci in range(CT):
                    nc.tensor.transpose(
                        tp[:, ci * 128:(ci + 1) * 128],
                        A16[:, k, ci * 128:(ci + 1) * 128],
                        ident[:],
                    )
                nc.scalar.copy(
                    AT16[:, :, k * 128:(k + 1) * 128],
                    tp[:].rearrange("p (ci n) -> p ci n", ci=CT),
                )
                # upper-triangle Gram matmuls for this chunk
                for mi in range(CT):
                    nc.tensor.matmul(
                        g_ps[mi],
                        A16[:, k, mi * 128:(mi + 1) * 128],
                        A16[:, k, mi * 128:],
                        start=(k == 0),
                        stop=(k == NT - 1),
                        # g1/g3 share a bank; per-element has_written makes
                        # disjoint-region groups safe on HW
                        skip_group_check=(mi % 2 == 1),
                    )
            k0 += gsz

        # G (upper) PSUM -> SBUF
        for mi in range(CT):
            if mi % 2 == 0:
                nc.vector.tensor_copy(G32[:, mi, mi * 128:], g_ps[mi])
            else:
                nc.scalar.copy(G32[:, mi, mi * 128:], g_ps[mi])
        # HAM bridge: the reconstruction/softmax window has no real matmuls
        # (transpose-mode doesn't count as PE-busy for the clock gate), so a
        # badly-aligned HAM window re-throttles the PE right before mm2.
        # A few dummy matmuls in this PE-idle stretch keep it at 2.4 GHz.
        bridge_ps = wps.tile([128, C], F32, name="bridge", tag="w")
        for wi in range(6):
            nc.tensor.matmul(
                bridge_ps[:], warm_src[:, 0:128], warm_src[:],
                start=(wi == 0), stop=(wi == 5),
            )
        # reconstruct lower triangle: G[mi, j] = G[j, mi]^T for j < mi
        for mi in range(1, CT):
            for j in range(mi):
                lb = wps.tile([128, 128], F32, name="lb", tag="w")
                nc.tensor.transpose(
                    lb[:], G32[:, j, mi * 128:(mi + 1) * 128], ident32[:])
                if (mi + j) % 2 == 0:
                    nc.vector.tensor_copy(G32[:, mi, j * 128:(j + 1) * 128], lb[:])
                else:
                    nc.scalar.copy(G32[:, mi, j * 128:(j + 1) * 128], lb[:])

        # softmax over rows of G (free axis)
        for mi in range(CT):
            nmax = stat.tile([128, 1], F32)
            nc.vector.tensor_reduce(
                nmax[:], G32[:, mi, :],
                axis=mybir.AxisListType.X, op=mybir.AluOpType.max, negate=True,
            )
            esum = stat.tile([128, 1], F32)
            nc.scalar.activation(
                E32[:, mi, :], G32[:, mi, :],
                mybir.ActivationFunctionType.Exp,
                bias=nmax[:], scale=1.0, accum_out=esum[:],
            )
            rsum = stat.tile([128, 1], F32)
            nc.vector.reciprocal(rsum[:], esum[:])
            nc.vector.tensor_scalar_mul(P16[:, mi, :], E32[:, mi, :], rsum[:])

        # Y = A @ P, epilogue out = gamma * Y + x
        out_groups = [OGRP] * (ONG - 1) + [1, 1]
        t0 = 0
        for h, osz in enumerate(out_groups):
            r0 = t0 * 128
            r1 = (t0 + osz) * 128
            o32 = ostage.tile([128, OGRP, C], F32)
            for j in range(osz):
                t = t0 + j
                y = wps.tile([128, C], F32, name="y", tag="w")
                for ci in range(CT):
                    nc.tensor.matmul(
                        y[:],
                        AT16[:, ci, t * 128:(t + 1) * 128],
                        P16[:, ci, :],
                        start=(ci == 0),
                        stop=(ci == CT - 1),
                    )
                nc.vector.scalar_tensor_tensor(
                    o32[:, j, :], y[:], gB[:], A32[:, t, :],
                    op0=mybir.AluOpType.mult, op1=mybir.AluOpType.add,
                )
            # last groups ride the idle ACT ring to dodge Sync-ring backlog
            oeng = nc.scalar if h >= len(out_groups) - 2 else nc.sync
            oeng.dma_start(
                out[r0:r1, :].rearrange("(t p) c -> p t c", p=128),
                o32[:, 0:osz, :],
            )
            t0 += osz


def build():
    nc = bacc.Bacc("TRN2", target_bir_lowering=False, debug=False)
    x = nc.dram_tensor("x", [HW, C], F32, kind="ExternalInput").ap()
    gamma = nc.dram_tensor("gamma", [128, 1], F32, kind="ExternalInput").ap()
    out = nc.dram_tensor("out", [HW, C], F32, kind="ExternalOutput").ap()
    with tile.TileContext(nc) as tc:
        _emit(nc, tc, out, x, gamma)
    nc.compile()
    return nc


def kernel(x: np.ndarray, gamma: np.ndarray, trace: bool = False):
    assert x.shape == (B, H, W, C), x.shape
    if "nc" not in _CACHE:
        _CACHE["nc"] = build()
    nc = _CACHE["nc"]

    g128 = np.full((128, 1), np.float32(np.asarray(gamma).reshape(-1)[0]),
                   dtype=np.float32)
    in_maps = [
        {
            "x": np.ascontiguousarray(
                np.asarray(x[i], dtype=np.float32).reshape(HW, C)),
            "gamma": g128,
        }
        for i in range(B)
    ]
    if trace:
        res = run_bass_kernel_spmd(nc, in_maps, core_ids=list(range(B)),
                                   trace=True)
    else:
        # Force-untraced: a stray BASS_TRACE in the environment would route
        # through profiling hooks this image may not have.
        import os
        prev = os.environ.get("BASS_NEVER_TRACE")
        os.environ["BASS_NEVER_TRACE"] = "1"
        try:
            res = run_bass_kernel_spmd(nc, in_maps, core_ids=list(range(B)))
        finally:
            if prev is None:
                os.environ.pop("BASS_NEVER_TRACE", None)
            else:
                os.environ["BASS_NEVER_TRACE"] = prev
    _CACHE["last_result"] = res
    out = np.stack([res.results[i]["out"] for i in range(B)], axis=0)
    return out.reshape(B, H, W, C).astype(np.float32)


# revision 32
# speedup vs baseline: 1.0086x; 1.0086x over previous
"""CAM (channel self-attention) kernel for Trainium2 — 8 NeuronCores, batch-parallel.

Math per batch element b (A = x[b] reshaped [N=4096, C=512]):
    G = A^T A                  [C, C]   (symmetric!)
    P = softmax_rows(G)        [C, C]
    Y = A P                    [N, C]
    out = gamma * Y + x

Sharding: data-parallel over batch — core i handles batch element i.
No cross-core communication needed.

Per-core schedule:
  - DMA x in 1 MiB groups -> A32 (f32, resident), cast to A16 (bf16).
  - Per 128-row chunk k, interleaved to keep the PE HAM-warm:
    cast -> 4 PE transposes (A^T blocks -> PSUM -> one strided copy to
    AT16) -> upper-triangle Gram matmuls (free dims 512/384/256/128,
    exploiting G's symmetry).
  - Lower triangle of G reconstructed with 6 PE transposes of the upper
    blocks after the Gram accumulation lands in SBUF.
  - softmax: DVE row-max (negated) -> ACT exp with fused row-sum -> DVE
    reciprocal -> DVE per-row scale, output bf16 P16.
  - Y = A P via PE: lhsT = AT16 tile, rhs = P16.
  - epilogue: one DVE scalar_tensor_tensor: out = (Y * gamma) + A32,
    staged in 512 KiB groups, DMA'd out.
"""

import numpy as np

import concourse.tile as tile
from concourse import bacc, mybir
from concourse.bass_utils import run_bass_kernel_spmd
from concourse.masks import make_identity

B = 8
H = 64
W = 64
C = 512
HW = H * W            # 4096 rows per batch element
NT = HW // 128        # 32 row chunks of 128
CT = C // 128         # 4 col chunks of 128
GRP = 4               # row chunks per input DMA group (1 MiB)
OGRP = 2              # row chunks per output DMA group (512 KiB)
ONG = NT // OGRP      # 16 output groups

F32 = mybir.dt.float32
BF16 = mybir.dt.bfloat16

_CACHE = {}


def _emit(nc, tc, out, x, gamma):
    from contextlib import ExitStack

    with ExitStack() as ctx:
        big = ctx.enter_context(tc.tile_pool(name="big", bufs=1))
        small = ctx.enter_context(tc.tile_pool(name="small", bufs=1))
        stat = ctx.enter_context(tc.tile_pool(name="stat", bufs=4))
        ostage = ctx.enter_context(tc.tile_pool(name="ostage", bufs=4))
        gps = ctx.enter_context(tc.tile_pool(name="gps", bufs=1, space="PSUM"))
        wps = ctx.enter_context(tc.tile_pool(name="wps", bufs=5, space="PSUM"))

        A32 = big.tile([128, NT, C], F32)     # x rows, n on partitions
        A16 = big.tile([128, NT, C], BF16)    # bf16 cast of A32
        AT16 = big.tile([128, CT, HW], BF16)  # A^T, c on partitions
        G32 = big.tile([128, CT, C], F32)     # full Gram matrix in SBUF
        E32 = big.tile([128, CT, C], F32)     # exp(G - rowmax)
        P16 = big.tile([128, CT, C], BF16)    # softmax(G) in bf16

        ident = small.tile([128, 128], BF16)
        make_identity(nc, ident[:])
        ident32 = small.tile([128, 128], F32)
        make_identity(nc, ident32[:])

        gB = small.tile([128, 1], F32)        # gamma broadcast to all partitions

        # PE warm-up: the HAM clock gate holds the PE at 1.2 GHz until it has
        # been busy ~3.4us. The PE is otherwise idle until the first input
        # chunk lands (~11us), so burn that window with dummy matmuls on a
        # zeroed scratch tile; real matmuls then start at 2.4 GHz.
        warm_src = small.tile([128, C], BF16)
        nc.gpsimd.memset(warm_src[:], 0.0)
        warm_ps = wps.tile([128, C], F32, name="warm", tag="w")
        for wi in range(30):
            nc.tensor.matmul(
                warm_ps[:], warm_src[:, 0:128], warm_src[:],
                start=(wi == 0), stop=(wi == 29),
            )

        # Upper-triangle Gram accumulators: G[mi-chunk, mi*128:].
        # g1 (384 cols) and g3 (128 cols) share one PSUM bank.
        g0 = gps.tile([128, C], F32, name="g0", tag="g0")
        g13 = gps.tile([128, C], F32, name="g13", tag="g13")
        g2 = gps.tile([128, C - 256], F32, name="g2", tag="g2")
        g_ps = [g0[:], g13[:, 0:384], g2[:], g13[:, 384:512]]

        # First loads chunk-granular so the PE can start early, then 1 MiB.
        load_groups = [1, 1, 2] + [GRP] * ((NT - 4) // GRP)
        assert sum(load_groups) == NT
        k0 = 0
        for gi, gsz in enumerate(load_groups):
            r0 = k0 * 128
            r1 = (k0 + gsz) * 128
            nc.sync.dma_start(
                A32[:, k0:k0 + gsz, :],
                x[r0:r1, :].rearrange("(t p) c -> p t c", p=128),
            )
            if gi == 0:
                # gamma: tiny load on the ACT HWDGE ring, off the input path
                nc.scalar.dma_start(gB[:], gamma[:])
            for j in range(gsz):
                k = k0 + j
                # cast f32 -> bf16 (DVE; keeps ACT free for A^T copies)
                nc.vector.tensor_copy(A16[:, k, :], A32[:, k, :])
                # A^T blocks of this chunk -> one PSUM bank, one strided copy
                tp = wps.tile([128, CT * 128], BF16, name="tp", tag="w")
                for ci in range(CT):
                    nc.tensor.transpose(
                        tp[:, ci * 128:(ci + 1) * 128],
                        A16[:, k, ci * 128:(ci + 1) * 128],
                        ident[:],
                    )
                nc.scalar.copy(
                    AT16[:, :, k * 128:(k + 1) * 128],
                    tp[:].rearrange("p (ci n) -> p ci n", ci=CT),
                )
                # upper-triangle Gram matmuls for this chunk
                for mi in range(CT):
                    nc.tensor.matmul(
                        g_ps[mi],
                        A16[:, k, mi * 128:(mi + 1) * 128],
                        A16[:, k, mi * 128:],
                        start=(k == 0),
                        stop=(k == NT - 1),
                        # g1/g3 share a bank; per-element has_written makes
                        # disjoint-region groups safe on HW
                        skip_group_check=(mi % 2 == 1),
                    )
            k0 += gsz

        # G (upper) PSUM -> SBUF
        for mi in range(CT):
            if mi % 2 == 0:
                nc.vector.tensor_copy(G32[:, mi, mi * 128:], g_ps[mi])
            else:
                nc.scalar.copy(G32[:, mi, mi * 128:], g_ps[mi])
        # reconstruct lower triangle: G[mi, j] = G[j, mi]^T for j < mi
        for mi in range(1, CT):
            for j in range(mi):
                lb = wps.tile([128, 128], F32, name="lb", tag="w")
                nc.tensor.transpose(
                    lb[:], G32[:, j, mi * 128:(mi + 1) * 128], ident32[:])
                if (mi + j) % 2 == 0:
                    nc.vector.tensor_copy(G32[:, mi, j * 128:(j + 1) * 128], lb[:])
                else:
                    nc.scalar.copy(G32[:, mi, j * 128:(j + 1) * 128], lb[:])

        # softmax over rows of G (free axis)
        for mi in range(CT):
            nmax = stat.tile([128, 1], F32)
            nc.vector.tensor_reduce(
                nmax[:], G32[:, mi, :],
                axis=mybir.AxisListType.X, op=mybir.AluOpType.max, negate=True,
            )
            esum = stat.tile([128, 1], F32)
            nc.scalar.activation(
                E32[:, mi, :], G32[:, mi, :],
                mybir.ActivationFunctionType.Exp,
                bias=nmax[:], scale=1.0, accum_out=esum[:],
            )
            rsum = stat.tile([128, 1], F32)
            nc.vector.reciprocal(rsum[:], esum[:])
            nc.vector.tensor_scalar_mul(P16[:, mi, :], E32[:, mi, :], rsum[:])

        # Y = A @ P, epilogue out = gamma * Y + x
        out_groups = [OGRP] * (ONG - 1) + [1, 1]
        t0 = 0
        for h, osz in enumerate(out_groups):
            r0 = t0 * 128
            r1 = (t0 + osz) * 128
            o32 = ostage.tile([128, OGRP, C], F32)
            for j in range(osz):
                t = t0 + j
                y = wps.tile([128, C], F32, name="y", tag="w")
                for ci in range(CT):
                    nc.tensor.matmul(
                        y[:],
                        AT16[:, ci, t * 128:(t + 1) * 128],
                        P16[:, ci, :],
                        start=(ci == 0),
                        stop=(ci == CT - 1),
                    )
                nc.vector.scalar_tensor_tensor(
                    o32[:, j, :], y[:], gB[:], A32[:, t, :],
                    op0=mybir.AluOpType.mult, op1=mybir.AluOpType.add,
                )
            # last groups ride the idle ACT ring to dodge Sync-ring backlog
            oeng = nc.scalar if h >= len(out_groups) - 2 else nc.sync
            oeng.dma_start(
                out[r0:r1, :].rearrange("(t p) c -> p t c", p=128),
                o32[:, 0:osz, :],
            )
            t0 += osz


def build():
    nc = bacc.Bacc("TRN2", target_bir_lowering=False, debug=False)
    x = nc.dram_tensor("x", [HW, C], F32, kind="ExternalInput").ap()
    gamma = nc.dram_tensor("gamma", [128, 1], F32, kind="ExternalInput").ap()
    out = nc.dram_tensor("out", [HW, C], F32, kind="ExternalOutput").ap()
    with tile.TileContext(nc) as tc:
        _emit(nc, tc, out, x, gamma)
    nc.compile()
    return nc


def kernel(x: np.ndarray, gamma: np.ndarray, trace: bool = False):
    assert x.shape == (B, H, W, C), x.shape
    if "nc" not in _CACHE:
        _CACHE["nc"] = build()
    nc = _CACHE["nc"]

    g128 = np.full((128, 1), np.float32(np.asarray(gamma).reshape(-1)[0]),
                   dtype=np.float32)
    in_maps = [
        {
            "x": np.ascontiguousarray(
                np.asarray(x[i], dtype=np.float32).reshape(HW, C)),
            "gamma": g128,
        }
        for i in range(B)
    ]
    if trace:
        res = run_bass_kernel_spmd(nc, in_maps, core_ids=list(range(B)),
                                   trace=True)
    else:
        # Force-untraced: a stray BASS_TRACE in the environment would route
        # through profiling hooks this image may not have.
        import os
        prev = os.environ.get("BASS_NEVER_TRACE")
        os.environ["BASS_NEVER_TRACE"] = "1"
        try:
            res = run_bass_kernel_spmd(nc, in_maps, core_ids=list(range(B)))
        finally:
            if prev is None:
                os.environ.pop("BASS_NEVER_TRACE", None)
            else:
                os.environ["BASS_NEVER_TRACE"] = prev
    _CACHE["last_result"] = res
    out = np.stack([res.results[i]["out"] for i in range(B)], axis=0)
    return out.reshape(B, H, W, C).astype(np.float32)


# revision 33
# speedup vs baseline: 1.0087x; 1.0001x over previous
"""CAM (channel self-attention) kernel for Trainium2 — 8 NeuronCores, batch-parallel.

Math per batch element b (A = x[b] reshaped [N=4096, C=512]):
    G = A^T A                  [C, C]   (symmetric!)
    P = softmax_rows(G)        [C, C]
    Y = A P                    [N, C]
    out = gamma * Y + x

Sharding: data-parallel over batch — core i handles batch element i.
No cross-core communication needed.

Per-core schedule:
  - DMA x in 1 MiB groups -> A32 (f32, resident), cast to A16 (bf16).
  - Per 128-row chunk k, interleaved to keep the PE HAM-warm:
    cast -> 4 PE transposes (A^T blocks -> PSUM -> one strided copy to
    AT16) -> upper-triangle Gram matmuls (free dims 512/384/256/128,
    exploiting G's symmetry).
  - Lower triangle of G reconstructed with 6 PE transposes of the upper
    blocks after the Gram accumulation lands in SBUF.
  - softmax: DVE row-max (negated) -> ACT exp with fused row-sum -> DVE
    reciprocal -> DVE per-row scale, output bf16 P16.
  - Y = A P via PE: lhsT = AT16 tile, rhs = P16.
  - epilogue: one DVE scalar_tensor_tensor: out = (Y * gamma) + A32,
    staged in 512 KiB groups, DMA'd out.
"""

import numpy as np

import concourse.tile as tile
from concourse import bacc, mybir
from concourse.bass_utils import run_bass_kernel_spmd
from concourse.masks import make_identity

B = 8
H = 64
W = 64
C = 512
HW = H * W            # 4096 rows per batch element
NT = HW // 128        # 32 row chunks of 128
CT = C // 128         # 4 col chunks of 128
GRP = 4               # row chunks per input DMA group (1 MiB)
OGRP = 2              # row chunks per output DMA group (512 KiB)
ONG = NT // OGRP      # 16 output groups

F32 = mybir.dt.float32
BF16 = mybir.dt.bfloat16

_CACHE = {}


def _emit(nc, tc, out, x, gamma):
    from contextlib import ExitStack

    with ExitStack() as ctx:
        big = ctx.enter_context(tc.tile_pool(name="big", bufs=1))
        small = ctx.enter_context(tc.tile_pool(name="small", bufs=1))
        stat = ctx.enter_context(tc.tile_pool(name="stat", bufs=4))
        ostage = ctx.enter_context(tc.tile_pool(name="ostage", bufs=4))
        gps = ctx.enter_context(tc.tile_pool(name="gps", bufs=1, space="PSUM"))
        wps = ctx.enter_context(tc.tile_pool(name="wps", bufs=5, space="PSUM"))

        A32 = big.tile([128, NT, C], F32)     # x rows, n on partitions
        A16 = big.tile([128, NT, C], BF16)    # bf16 cast of A32
        AT16 = big.tile([128, CT, HW], BF16)  # A^T, c on partitions
        G32 = big.tile([128, CT, C], F32)     # full Gram matrix in SBUF
        E32 = big.tile([128, CT, C], F32)     # exp(G - rowmax)
        P16 = big.tile([128, CT, C], BF16)    # softmax(G) in bf16

        ident = small.tile([128, 128], BF16)
        make_identity(nc, ident[:])
        ident32 = small.tile([128, 128], F32)
        make_identity(nc, ident32[:])

        gB = small.tile([128, 1], F32)        # gamma broadcast to all partitions

        # PE warm-up: the HAM clock gate holds the PE at 1.2 GHz until it has
        # been busy ~3.4us. The PE is otherwise idle until the first input
        # chunk lands (~11us), so burn that window with dummy matmuls on a
        # zeroed scratch tile; real matmuls then start at 2.4 GHz.
        warm_src = small.tile([128, C], BF16)
        nc.gpsimd.memset(warm_src[:], 0.0)
        warm_ps = wps.tile([128, C], F32, name="warm", tag="w")
        for wi in range(30):
            nc.tensor.matmul(
                warm_ps[:], warm_src[:, 0:128], warm_src[:],
                start=(wi == 0), stop=(wi == 29),
            )

        # Upper-triangle Gram accumulators: G[mi-chunk, mi*128:].
        # g1 (384 cols) and g3 (128 cols) share one PSUM bank.
        g0 = gps.tile([128, C], F32, name="g0", tag="g0")
        g13 = gps.tile([128, C], F32, name="g13", tag="g13")
        g2 = gps.tile([128, C - 256], F32, name="g2", tag="g2")
        g_ps = [g0[:], g13[:, 0:384], g2[:], g13[:, 384:512]]

        # First loads chunk-granular so the PE can start early, then 1 MiB.
        load_groups = [1, 1, 2] + [GRP] * ((NT - 4) // GRP)
        assert sum(load_groups) == NT
        k0 = 0
        for gi, gsz in enumerate(load_groups):
            r0 = k0 * 128
            r1 = (k0 + gsz) * 128
            nc.sync.dma_start(
                A32[:, k0:k0 + gsz, :],
                x[r0:r1, :].rearrange("(t p) c -> p t c", p=128),
            )
            if gi == 0:
                # gamma: tiny load on the ACT HWDGE ring, off the input path
                nc.scalar.dma_start(gB[:], gamma[:])
            for j in range(gsz):
                k = k0 + j
                # cast f32 -> bf16 (DVE; keeps ACT free for A^T copies)
                nc.vector.tensor_copy(A16[:, k, :], A32[:, k, :])
                # A^T blocks of this chunk -> one PSUM bank, one strided copy
                tp = wps.tile([128, CT * 128], BF16, name="tp", tag="w")
                for ci in range(CT):
                    nc.tensor.transpose(
                        tp[:, ci * 128:(ci + 1) * 128],
                        A16[:, k, ci * 128:(ci + 1) * 128],
                        ident[:],
                    )
                nc.scalar.copy(
                    AT16[:, :, k * 128:(k + 1) * 128],
                    tp[:].rearrange("p (ci n) -> p ci n", ci=CT),
                )
                # upper-triangle Gram matmuls for this chunk
                for mi in range(CT):
                    nc.tensor.matmul(
                        g_ps[mi],
                        A16[:, k, mi * 128:(mi + 1) * 128],
                        A16[:, k, mi * 128:],
                        start=(k == 0),
                        stop=(k == NT - 1),
                        # g1/g3 share a bank; per-element has_written makes
                        # disjoint-region groups safe on HW
                        skip_group_check=(mi % 2 == 1),
                    )
            k0 += gsz

        # G (upper) PSUM -> SBUF
        for mi in range(CT):
            if mi % 2 == 0:
                nc.vector.tensor_copy(G32[:, mi, mi * 128:], g_ps[mi])
            else:
                nc.scalar.copy(G32[:, mi, mi * 128:], g_ps[mi])
        # reconstruct lower triangle: G[mi, j] = G[j, mi]^T for j < mi
        for mi in range(1, CT):
            for j in range(mi):
                lb = wps.tile([128, 128], F32, name="lb", tag="w")
                nc.tensor.transpose(
                    lb[:], G32[:, j, mi * 128:(mi + 1) * 128], ident32[:])
                if (mi + j) % 2 == 0:
                    nc.vector.tensor_copy(G32[:, mi, j * 128:(j + 1) * 128], lb[:])
                else:
                    nc.scalar.copy(G32[:, mi, j * 128:(j + 1) * 128], lb[:])

        # softmax over rows of G (free axis)
        for mi in range(CT):
            nmax = stat.tile([128, 1], F32)
            nc.vector.tensor_reduce(
                nmax[:], G32[:, mi, :],
                axis=mybir.AxisListType.X, op=mybir.AluOpType.max, negate=True,
            )
            esum = stat.tile([128, 1], F32)
            nc.scalar.activation(
                E32[:, mi, :], G32[:, mi, :],
                mybir.ActivationFunctionType.Exp,
                bias=nmax[:], scale=1.0, accum_out=esum[:],
            )
            rsum = stat.tile([128, 1], F32)
            nc.vector.reciprocal(rsum[:], esum[:])
            nc.vector.tensor_scalar_mul(P16[:, mi, :], E32[:, mi, :], rsum[:])

        # Y = A @ P, epilogue out = gamma * Y + x
        out_groups = [OGRP] * (ONG - 1) + [1, 1]
        t0 = 0
        for h, osz in enumerate(out_groups):
            r0 = t0 * 128
            r1 = (t0 + osz) * 128
            o32 = ostage.tile([128, OGRP, C], F32)
            for j in range(osz):
                t = t0 + j
                y = wps.tile([128, C], F32, name="y", tag="w")
                for ci in range(CT):
                    nc.tensor.matmul(
                        y[:],
                        AT16[:, ci, t * 128:(t + 1) * 128],
                        P16[:, ci, :],
                        start=(ci == 0),
                        stop=(ci == CT - 1),
                    )
                nc.vector.scalar_tensor_tensor(
                    o32[:, j, :], y[:], gB[:], A32[:, t, :],
                    op0=mybir.AluOpType.mult, op1=mybir.AluOpType.add,
                )
            # last groups ride the idle ACT ring to dodge Sync-ring backlog
            oeng = nc.scalar if h >= len(out_groups) - 2 else nc.sync
            oeng.dma_start(
                out[r0:r1, :].rearrange("(t p) c -> p t c", p=128),
                o32[:, 0:osz, :],
            )
            t0 += osz


def build():
    nc = bacc.Bacc("TRN2", target_bir_lowering=False, debug=False)
    x = nc.dram_tensor("x", [HW, C], F32, kind="ExternalInput").ap()
    gamma = nc.dram_tensor("gamma", [128, 1], F32, kind="ExternalInput").ap()
    out = nc.dram_tensor("out", [HW, C], F32, kind="ExternalOutput").ap()
    with tile.TileContext(nc, pool_alloc_mode='queue') as tc:
        _emit(nc, tc, out, x, gamma)
    nc.compile()
    return nc


def kernel(x: np.ndarray, gamma: np.ndarray, trace: bool = False):
    assert x.shape == (B, H, W, C), x.shape
    if "nc" not in _CACHE:
        _CACHE["nc"] = build()
    nc = _CACHE["nc"]

    g128 = np.full((128, 1), np.float32(np.asarray(gamma).reshape(-1)[0]),
                   dtype=np.float32)
    in_maps = [
        {
            "x": np.ascontiguousarray(
                np.asarray(x[i], dtype=np.float32).reshape(HW, C)),
            "gamma": g128,
        }
        for i in range(B)
    ]
    if trace:
        res = run_bass_kernel_spmd(nc, in_maps, core_ids=list(range(B)),
                                   trace=True)
    else:
        # Force-untraced: a stray BASS_TRACE in the environment would route
        # through profiling hooks this image may not have.
        import os
        prev = os.environ.get("BASS_NEVER_TRACE")
        os.environ["BASS_NEVER_TRACE"] = "1"
        try:
            res = run_bass_kernel_spmd(nc, in_maps, core_ids=list(range(B)))
        finally:
            if prev is None:
                os.environ.pop("BASS_NEVER_TRACE", None)
            else:
                os.environ["BASS_NEVER_TRACE"] = prev
    _CACHE["last_result"] = res
    out = np.stack([res.results[i]["out"] for i in range(B)], axis=0)
    return out.reshape(B, H, W, C).astype(np.float32)
